# revision 20
# baseline (speedup 1.0000x reference)
"""4-layer GAT (GNN message passing) on 8 TRN2 NeuronCores.

Strategy (graph/data parallel per sharding hint):
- Nodes are permuted into 392 degree-balanced blocks of 128 (49 blocks/core).
- Node phase (sharded): LN (folded into weights) -> bf16 projection matmul
  with attention-logit columns folded in -> per-core table shard
  [msg(512) | al_src(8) | pad to 640] bf16, AllGathered to all cores.
- Edge phase (dst-sharded): per dst-block, TWO InstDMAGatherAnt gathers
  (table halves, int16 idx limit) fetch all source rows in 2 instructions
  (SWDGE fixed overhead ~1us/instr makes per-chunk indirect DMA untenable).
  One-hot S built on-device via is_equal; dst logits broadcast to edges via
  transpose(S) matmul; softmax without segment-max (exactly cancels);
  attention-weighted scatter-add via one-hot matmuls into PSUM.
- Final layer output + residual written per-core; host inverse-permutes.
"""
import heapq
import ml_dtypes
import numpy as np

from concourse import bass, bacc, mybir, tile
from concourse.bass_utils import run_bass_kernel_spmd
from concourse.masks import make_identity

F32 = mybir.dt.float32
BF16 = mybir.dt.bfloat16
I16 = mybir.dt.int16
AF = mybir.ActivationFunctionType
AX = mybir.AxisListType
OP = mybir.AluOpType

N = 50000
IN = 128
OUT = 3
NEG_SLOPE = 0.2
EPS = 1e-6
P = 128
NCORES = 8
NB = 49                    # dst blocks per core
NPC = NB * P               # 6272 nodes per core
NTOT = NCORES * NPC        # 50176 padded nodes
PADV = 200.0               # didx value for padding edge slots (no dst match)
# The full table is split into two Shared tensors (A: blocks 0..25, B: blocks
# 26..48 of every core) so the AllGather can be chunked (Shared DRAM tensors
# allow a single writing instruction) and dma_gather's int16 row indices fit.
SPLIT = 26
ROWS_A = SPLIT * NCORES * P          # 26624
ROWS_B = (NB - SPLIT) * NCORES * P   # 23552

# layer configs: (D_in, D_out(heads*co), heads, co, table_cols)
LAYERS = [(IN, 512, 8, 64, 640), (512, 512, 8, 64, 640),
          (512, 512, 8, 64, 640), (512, 3, 1, 3, 128)]


# ----------------------------------------------------------------- host prep

def _build_partition(edge_index):
    src0 = edge_index[0].astype(np.int64)
    dst0 = edge_index[1].astype(np.int64)
    loops = np.arange(N, dtype=np.int64)
    src = np.concatenate([src0, loops])
    dst = np.concatenate([dst0, loops])
    deg = np.bincount(dst, minlength=N)

    nblocks = NCORES * NB
    order = np.argsort(-deg, kind="stable")
    heap = [(0, 0, b) for b in range(nblocks)]
    heapq.heapify(heap)
    blk_of = np.empty(N, np.int64)
    slot_of = np.empty(N, np.int64)
    spill = []
    for v in order:
        while True:
            load, cnt, b = heapq.heappop(heap)
            if cnt < P:
                break
            spill.append((load, cnt, b))
        blk_of[v] = b
        slot_of[v] = cnt
        heapq.heappush(heap, (load + int(deg[v]), cnt + 1, b))
        for item in spill:
            heapq.heappush(heap, item)
        spill.clear()

    new_id = blk_of * P + slot_of
    filled = np.zeros(NTOT, bool)
    filled[new_id] = True
    pad_ids = np.nonzero(~filled)[0]

    # table-row numbering inside the A/B Shared tensors
    bl_all = (new_id // P) % NB
    c_all = new_id // (NB * P)
    slot_all = new_id % P
    in_a = bl_all < SPLIT
    row_tab = np.where(
        in_a,
        (c_all * SPLIT + bl_all) * P + slot_all,
        (c_all * (NB - SPLIT) + (bl_all - SPLIT)) * P + slot_all)
    nsrc = np.where(in_a[src], row_tab[src], row_tab[src] + ROWS_A)
    ndst = new_id[dst]
    # dummy edges so padded dst slots get a nonzero softmax denominator
    nsrc = np.concatenate([nsrc, np.zeros(len(pad_ids), np.int64)])
    ndst = np.concatenate([ndst, pad_ids])

    eblk = ndst // P
    order_e = np.lexsort((ndst, eblk))
    nsrc, ndst, eblk = nsrc[order_e], ndst[order_e], eblk[order_e]
    starts_all = np.searchsorted(eblk, np.arange(nblocks + 1))

    # per-block A/B split (src < HALF vs >=) for int16 dma_gather indices
    blocks = []
    ca_max = cb_max = 0
    for b in range(nblocks):
        lo, hi = starts_all[b], starts_all[b + 1]
        bs = nsrc[lo:hi]
        bd = ndst[lo:hi] - b * P
        isa = bs < ROWS_A
        sa, da = bs[isa], bd[isa]
        sb_, db = bs[~isa] - ROWS_A, bd[~isa]
        ca = max(1, (len(sa) + P - 1) // P)
        cb = max(1, (len(sb_) + P - 1) // P)
        ca_max, cb_max = max(ca_max, ca), max(cb_max, cb)
        blocks.append((sa, da, sb_, db))

    CA, CB = ca_max, cb_max
    NCHB = CA + CB
    gidx = np.zeros((NCORES, P, NB * NCHB * 8), np.int16)
    didx = np.full((NCORES, P, NB * NCHB), PADV, np.float32)

    def wrap16(seq, cnum):
        out = np.zeros(cnum * P, np.int16)
        out[:len(seq)] = seq.astype(np.int16)
        # [16, cnum*8] wrapped, replicated to all 8 Q7-core stripes
        return np.tile(out.reshape(cnum * 8, 16).T, (8, 1))

    def slots(seq, cnum):
        out = np.full(cnum * P, PADV, np.float32)
        out[:len(seq)] = seq.astype(np.float32)
        return out.reshape(cnum, P).T             # [P, cnum]

    for b in range(nblocks):
        c, bl = divmod(b, NB)
        sa, da, sb_, db = blocks[b]
        col = bl * NCHB * 8
        gidx[c, :, col:col + CA * 8] = wrap16(sa, CA)
        gidx[c, :, col + CA * 8:col + NCHB * 8] = wrap16(sb_, CB)
        dcol = bl * NCHB
        didx[c, :, dcol:dcol + CA] = slots(da, CA)
        didx[c, :, dcol + CA:dcol + NCHB] = slots(db, CB)
    return new_id, gidx, didx, CA, CB


def _fold_weights(inputs):
    ws = {}
    for i, (ci, D, h, co, tc) in enumerate(LAYERS):
        g = np.asarray(inputs[f'ln_g{i}'], np.float32)
        b = np.asarray(inputs[f'ln_b{i}'], np.float32)
        W = np.asarray(inputs[f'W{i}'], np.float32)
        asr = np.asarray(inputs[f'asr{i}'], np.float32)       # [h, co]
        adt = np.asarray(inputs[f'adt{i}'], np.float32)
        wp = np.ascontiguousarray((W * g[None, :]).T)         # [ci, D] f32
        cv = (W @ b).astype(np.float32)                       # [D]
        wp3 = wp.reshape(ci, h, co)
        wa = np.concatenate([
            np.einsum('ihc,hc->ih', wp3, asr),
            np.einsum('ihc,hc->ih', wp3, adt)], axis=1)       # [ci, 2h]
        c0 = np.concatenate([
            np.einsum('hc,hc->h', cv.reshape(h, co), asr),
            np.einsum('hc,hc->h', cv.reshape(h, co), adt)])   # [2h]
        ws[f'wp{i}'] = wp.astype(ml_dtypes.bfloat16)
        ws[f'wa{i}'] = np.ascontiguousarray(wa).astype(ml_dtypes.bfloat16)
        ws[f'cv{i}'] = cv.reshape(1, D)
        ws[f'c0{i}'] = c0.reshape(1, 2 * h).astype(np.float32)
        ws[f'bias{i}'] = np.asarray(inputs[f'b{i}'], np.float32).reshape(1, D)
    ws['res_wt'] = np.ascontiguousarray(
        np.asarray(inputs['res_W'], np.float32).T).astype(ml_dtypes.bfloat16)
    ws['res_b'] = np.asarray(inputs['res_b'], np.float32).reshape(1, OUT)
    ws['iota'] = np.arange(P, dtype=np.float32).reshape(1, P)
    return ws


# -------------------------------------------------------------- bass program

_DBG = False


def _dump(nc, pool, dst, src_ap, cols, dtype=F32):
    t = pool.tile([P, cols], dtype, name="dbgt")
    nc.vector.tensor_copy(out=t[:], in_=src_ap)
    nc.sync.dma_start(out=dst[:], in_=t[:])


def _build_program(CA, CB):
    nc = bacc.Bacc("TRN2", target_bir_lowering=False, debug=False,
                   num_devices=NCORES)
    NCHB = CA + CB

    x_s = nc.dram_tensor("x_s", [NPC, IN], F32, kind="ExternalInput")
    gidx_d = nc.dram_tensor("gidx", [P, NB * NCHB * 8], I16, kind="ExternalInput")
    didx_d = nc.dram_tensor("didx", [P, NB * NCHB], F32, kind="ExternalInput")
    wdram = {}
    for i, (ci, D, h, co, tc) in enumerate(LAYERS):
        wdram[f'wp{i}'] = nc.dram_tensor(f"wp{i}", [ci, D], BF16, kind="ExternalInput")
        wdram[f'wa{i}'] = nc.dram_tensor(f"wa{i}", [ci, 2 * h], BF16, kind="ExternalInput")
        wdram[f'cv{i}'] = nc.dram_tensor(f"cv{i}", [1, D], F32, kind="ExternalInput")
        wdram[f'c0{i}'] = nc.dram_tensor(f"c0{i}", [1, 2 * h], F32, kind="ExternalInput")
        wdram[f'bias{i}'] = nc.dram_tensor(f"bias{i}", [1, D], F32, kind="ExternalInput")
    wdram['res_wt'] = nc.dram_tensor("res_wt", [IN, OUT], BF16, kind="ExternalInput")
    wdram['res_b'] = nc.dram_tensor("res_b", [1, OUT], F32, kind="ExternalInput")
    wdram['iota'] = nc.dram_tensor("iota", [1, P], F32, kind="ExternalInput")
    out_ext = nc.dram_tensor("out", [NPC, OUT], F32, kind="ExternalOutput")
    dbg = {}
    if _DBG:
        dbg['shard0'] = nc.dram_tensor("dbg_shard0", [NPC, 640], BF16,
                                       kind="ExternalOutput")

        dbg['g0'] = nc.dram_tensor("dbg_g0", [P, (CA + CB) * 640], BF16,
                                   kind="ExternalOutput")
        dbg['s0'] = nc.dram_tensor("dbg_s0", [P, (CA + CB) * P], BF16,
                                   kind="ExternalOutput")
        dbg['ade0'] = nc.dram_tensor("dbg_ade0", [P, (CA + CB) * 8], F32,
                                     kind="ExternalOutput")
        dbg['pa0'] = nc.dram_tensor("dbg_pa0", [P, 512], F32,
                                    kind="ExternalOutput")
        dbg['pb0'] = nc.dram_tensor("dbg_pb0", [P, 16], F32,
                                    kind="ExternalOutput")
        dbg['h0'] = nc.dram_tensor("dbg_h0", [NPC, 512], F32,
                                   kind="ExternalOutput")

    rg = [list(range(NCORES))]

    with tile.TileContext(nc) as tc_:
        with (
            tc_.tile_pool(name="dram", bufs=1, space="DRAM") as dpool,
            tc_.tile_pool(name="const", bufs=1) as cpool,
            tc_.tile_pool(name="node", bufs=2) as npool,
            tc_.tile_pool(name="stats", bufs=3) as spool,
            tc_.tile_pool(name="edge", bufs=2) as epool,
            tc_.tile_pool(name="big", bufs=2) as bpool,
            tc_.tile_pool(name="psA", bufs=2, space="PSUM") as psA,
            tc_.tile_pool(name="psB", bufs=2, space="PSUM") as psB,
            tc_.tile_pool(name="psT", bufs=2, space="PSUM") as psT,
        ):
            shard, fullA, fullB = {}, {}, {}
            for li, (ci, D, h, co, tcl) in enumerate(LAYERS):
                shard[li] = dpool.tile([NPC, tcl], BF16, name=f"shard{li}")
                fullA[li] = dpool.tile([ROWS_A, tcl], BF16, name=f"fullA{li}",
                                       addr_space="Shared")
                fullB[li] = dpool.tile([ROWS_B, tcl], BF16, name=f"fullB{li}",
                                       addr_space="Shared")

            # --- constants
            ident = cpool.tile([P, P], F32, name="ident")
            make_identity(nc, ident[:])
            ident_bf = cpool.tile([P, P], BF16, name="ident_bf")
            nc.vector.tensor_copy(out=ident_bf[:], in_=ident[:])
            gidx = cpool.tile([P, NB * NCHB * 8], I16, name="gidx")
            nc.sync.dma_start(out=gidx[:], in_=gidx_d[:])
            didx = cpool.tile([P, NB * NCHB], F32, name="didx")
            nc.sync.dma_start(out=didx[:], in_=didx_d[:])
            iota_f = cpool.tile([P, P], F32, name="iota_f")
            nc.sync.dma_start(out=iota_f[:], in_=wdram['iota'][:].to_broadcast([P, P]))
            wsb = {}
            for i, (ci, D, h, co, tcl) in enumerate(LAYERS):
                kc = ci // P
                wt = cpool.tile([P, kc * D], BF16, name=f"wsb{i}")
                nc.sync.dma_start(
                    out=wt[:].rearrange("p (k d) -> p k d", k=kc),
                    in_=wdram[f'wp{i}'][:].rearrange("(k p) d -> p k d", p=P))
                wsb[f'wp{i}'] = wt
                wat = cpool.tile([P, kc * 2 * h], BF16, name=f"wa{i}sb")
                nc.sync.dma_start(
                    out=wat[:].rearrange("p (k d) -> p k d", k=kc),
                    in_=wdram[f'wa{i}'][:].rearrange("(k p) d -> p k d", p=P))
                wsb[f'wa{i}'] = wat
                for nm, w in (("cv", D), ("c0", 2 * h), ("bias", D)):
                    vt = cpool.tile([P, w], F32, name=f"{nm}{i}sb")
                    nc.sync.dma_start(
                        out=vt[:], in_=wdram[f'{nm}{i}'][:].to_broadcast([P, w]))
                    wsb[f'{nm}{i}'] = vt
            res_wt = cpool.tile([IN, OUT], BF16, name="res_wt_sb")
            nc.sync.dma_start(out=res_wt[:], in_=wdram['res_wt'][:])
            res_b = cpool.tile([P, OUT], F32, name="res_b_sb")
            nc.sync.dma_start(out=res_b[:],
                              in_=wdram['res_b'][:].to_broadcast([P, OUT]))
            res_sb = cpool.tile([P, NB * OUT], F32, name="res_sb")
            adst_sb = cpool.tile([P, NB * 8], BF16, name="adst_sb")
            h_sb = cpool.tile([P, NB * 512], BF16, name="h_sb")
            eps_t = cpool.tile([P, 1], F32, name="eps_t")
            nc.gpsimd.memset(eps_t[:], EPS)

            # ---------------- per-block node compute ----------------
            def node_block(li, t):
                ci, D, h, co, tcl = LAYERS[li]
                kc = ci // P
                rows = slice(t * P, (t + 1) * P)
                if li == 0:
                    ht = npool.tile([P, ci], F32, name="ht", tag="ht")
                    nc.sync.dma_start(out=ht[:], in_=x_s[rows, :])
                    xt_ps = psT.tile([P, P], BF16, name="xt_ps", tag="tp")
                    zx = npool.tile([P, P], BF16, name="zx", tag="zx")
                    nc.vector.tensor_copy(out=zx[:], in_=ht[:])
                    nc.tensor.transpose(out=xt_ps[:], in_=zx[:],
                                        identity=ident_bf[:])
                    xt_sb = npool.tile([P, P], BF16, name="xt_sb", tag="xt")
                    nc.vector.tensor_copy(out=xt_sb[:], in_=xt_ps[:])
                    res_ps = psB.tile([P, 16], F32, name="res_ps", tag="mmB")
                    nc.tensor.matmul(out=res_ps[:, :OUT], lhsT=xt_sb[:],
                                     rhs=res_wt[:], start=True, stop=True)
                    nc.vector.tensor_tensor(
                        out=res_sb[:, t * OUT:(t + 1) * OUT],
                        in0=res_ps[:, :OUT], in1=res_b[:], op=OP.add)
                else:
                    ht = None

                hin = ht[:] if li == 0 else h_sb[:, t * 512:t * 512 + ci]
                # layernorm (gamma/beta folded into wp/cv)
                mu_n = spool.tile([P, 1], F32, name="mu_n", tag="st1")
                nc.vector.reduce_sum(out=mu_n[:], in_=hin, axis=AX.X)
                nc.vector.tensor_scalar_mul(mu_n[:], mu_n[:], -1.0 / ci)
                sq = npool.tile([P, ci], F32, name="sq", tag="sq")
                ssq = spool.tile([P, 1], F32, name="ssq", tag="st2")
                nc.scalar.activation(out=sq[:], in_=hin, func=AF.Square,
                                     bias=mu_n[:, :1], accum_out=ssq[:, :1])
                std = spool.tile([P, 1], F32, name="std", tag="st3")
                nc.scalar.activation(out=std[:], in_=ssq[:], func=AF.Sqrt,
                                     scale=1.0 / ci, bias=eps_t[:, :1])
                rstd = spool.tile([P, 1], F32, name="rstd", tag="st4")
                nc.vector.reciprocal(out=rstd[:], in_=std[:])
                mur = spool.tile([P, 1], F32, name="mur", tag="st5")
                nc.vector.tensor_tensor(out=mur[:], in0=mu_n[:], in1=rstd[:],
                                        op=OP.mult)
                z = npool.tile([P, ci], BF16, name="z", tag="z")
                nc.scalar.activation(out=z[:], in_=hin, func=AF.Identity,
                                     scale=rstd[:, :1], bias=mur[:, :1])

                # projection (+ folded logit columns)
                pp = psA.tile([P, 512], F32, name="pp", tag="mmA")
                pl = psB.tile([P, 16], F32, name="pl", tag="mmB")
                wt = wsb[f'wp{li}']
                wat = wsb[f'wa{li}']
                for k in range(kc):
                    zt_ps = psT.tile([P, P], BF16, name="zt_ps", tag="tp")
                    nc.tensor.transpose(out=zt_ps[:], in_=z[:, k * P:(k + 1) * P],
                                        identity=ident_bf[:])
                    zt_sb = npool.tile([P, P], BF16, name="zt_sb", tag="zt")
                    nc.vector.tensor_copy(out=zt_sb[:], in_=zt_ps[:])
                    nc.tensor.matmul(out=pp[:, :D], lhsT=zt_sb[:],
                                     rhs=wt[:, k * D:(k + 1) * D],
                                     start=(k == 0), stop=(k == kc - 1))
                    nc.tensor.matmul(out=pl[:, :2 * h], lhsT=zt_sb[:],
                                     rhs=wat[:, k * 2 * h:(k + 1) * 2 * h],
                                     start=(k == 0), stop=(k == kc - 1))

                ptile = npool.tile([P, tcl], BF16, name="ptile", tag="ptb")
                nc.vector.tensor_tensor(
                    out=ptile[:, 0:D], in0=pp[:, :D],
                    in1=wsb[f'cv{li}'][:], op=OP.add)
                lsum = npool.tile([P, 2 * h], F32, name="lsum", tag="lsum")
                nc.vector.tensor_tensor(
                    out=lsum[:], in0=pl[:, :2 * h],
                    in1=wsb[f'c0{li}'][:], op=OP.add)
                nc.scalar.copy(out=ptile[:, D:D + h], in_=lsum[:, 0:h])
                nc.scalar.copy(out=adst_sb[:, t * 8:t * 8 + h],
                               in_=lsum[:, h:2 * h])
                nc.sync.dma_start(out=shard[li][rows, :], in_=ptile[:])

            def ag_group(li, j):
                if j == 0:
                    rows = slice(0, SPLIT * P)
                    ftab = fullA[li]
                else:
                    rows = slice(SPLIT * P, NB * P)
                    ftab = fullB[li]
                nc.gpsimd.collective_compute(
                    "AllGather", OP.bypass, replica_groups=rg,
                    ins=[shard[li][rows, :].opt()], outs=[ftab[:].opt()])

            # ---------------- per-block edge compute ----------------
            def edge_block(li, bl):
                ci, D, h, co, tcl = LAYERS[li]
                rows = slice(bl * P, (bl + 1) * P)
                g = bpool.tile([P, NCHB * tcl], BF16, name="g", tag="g")
                col = bl * NCHB * 8
                nc.gpsimd.dma_gather(
                    out_ap=g[:, 0:CA * tcl].rearrange("p (c t) -> p c t", c=CA),
                    in_ap=fullA[li][:],
                    idxs_ap=gidx[:, col:col + CA * 8],
                    num_idxs=CA * P, num_idxs_reg=CA * P, elem_size=tcl)
                nc.gpsimd.dma_gather(
                    out_ap=g[:, CA * tcl:].rearrange("p (c t) -> p c t", c=CB),
                    in_ap=fullB[li][:],
                    idxs_ap=gidx[:, col + CA * 8:col + NCHB * 8],
                    num_idxs=CB * P, num_idxs_reg=CB * P, elem_size=tcl)
                gv = g[:].rearrange("p (c t) -> p c t", c=NCHB)

                # one-hot S for all chunks (on-device, from didx)
                dcol = bl * NCHB
                s_all = epool.tile([P, NCHB * P], BF16, name="s_all", tag="S")
                sv = s_all[:].rearrange("p (c d) -> p c d", c=NCHB)
                nc.vector.tensor_tensor(
                    out=sv,
                    in0=didx[:, dcol:dcol + NCHB].unsqueeze(2)
                        .broadcast_to([P, NCHB, P]),
                    in1=iota_f[:].unsqueeze(1).broadcast_to([P, NCHB, P]),
                    op=OP.is_equal)

                # dst logits broadcast to edges: ade = S^T-matmul
                ade = psB.tile([P, NCHB * 8], F32, name="ade", tag="adps")
                for ch in range(NCHB):
                    t_ps = psT.tile([P, P], BF16, name="t_ps", tag="tp")
                    nc.tensor.transpose(out=t_ps[:],
                                        in_=s_all[:, ch * P:(ch + 1) * P],
                                        identity=ident_bf[:])
                    t_sb = epool.tile([P, P], BF16, name="t_sb", tag="T")
                    if ch % 2 == 0:
                        nc.scalar.copy(out=t_sb[:], in_=t_ps[:])
                    else:
                        nc.vector.tensor_copy(out=t_sb[:], in_=t_ps[:])
                    nc.tensor.matmul(
                        out=ade[:, ch * 8:ch * 8 + h], lhsT=t_sb[:],
                        rhs=adst_sb[:, bl * 8:bl * 8 + h],
                        start=True, stop=True)

                if _DBG and li == 0 and bl == 0:
                    nc.sync.dma_start(out=dbg['g0'][:], in_=g[:])
                    nc.sync.dma_start(out=dbg['s0'][:], in_=s_all[:])
                    _dump(nc, npool, dbg['ade0'], ade[:], NCHB * 8)

                # e = prelu(al_src + al_dst); exp(e) -> g logit slot
                eraw = epool.tile([P, NCHB * h], F32, name="eraw", tag="eraw")
                nc.vector.tensor_tensor(
                    out=eraw[:].rearrange("p (c h) -> p c h", c=NCHB),
                    in0=gv[:, :, D:D + h],
                    in1=ade[:].rearrange("p (c h) -> p c h", c=NCHB)[:, :, 0:h],
                    op=OP.add)
                el = epool.tile([P, NCHB * h], F32, name="el", tag="el")
                nc.scalar.activation(out=el[:], in_=eraw[:], func=AF.Prelu,
                                     alpha=NEG_SLOPE)
                nc.scalar.activation(
                    out=gv[:, :, D:D + h],
                    in_=el[:].rearrange("p (c h) -> p c h", c=NCHB),
                    func=AF.Exp)

                # weight messages in place per chunk
                for ch in range(NCHB):
                    ms = g[:, ch * tcl:ch * tcl + D]
                    nc.vector.tensor_tensor(
                        out=ms.rearrange("p (h c) -> p h c", h=h),
                        in0=ms.rearrange("p (h c) -> p h c", h=h),
                        in1=g[:, ch * tcl + D:ch * tcl + D + h]
                            .unsqueeze(2).broadcast_to([P, h, co]),
                        op=OP.mult)

                # scatter-add via one-hot matmuls
                pa = psA.tile([P, 512], F32, name="pa", tag="mmA")
                pb = psB.tile([P, 16], F32, name="pb", tag="mmB")
                for ch in range(NCHB):
                    nc.tensor.matmul(
                        out=pa[:, :D], lhsT=s_all[:, ch * P:(ch + 1) * P],
                        rhs=g[:, ch * tcl:ch * tcl + D],
                        start=(ch == 0), stop=(ch == NCHB - 1))
                    nc.tensor.matmul(
                        out=pb[:, :h], lhsT=s_all[:, ch * P:(ch + 1) * P],
                        rhs=g[:, ch * tcl + D:ch * tcl + D + h],
                        start=(ch == 0), stop=(ch == NCHB - 1))
                if _DBG and li == 0 and bl == 0:
                    _dump(nc, npool, dbg['pa0'], pa[:, :512], 512)
                    _dump(nc, npool, dbg['pb0'], pb[:, :16], 16)
                rs = spool.tile([P, h], F32, name="rs", tag="rs")
                nc.vector.reciprocal(out=rs[:], in_=pb[:, :h])
                gat = npool.tile([P, D], F32, name="gat", tag="gat")
                if h > 1:
                    nc.vector.tensor_tensor(
                        out=gat[:].rearrange("p (h c) -> p h c", h=h),
                        in0=pa[:, :D].rearrange("p (h c) -> p h c", h=h),
                        in1=rs[:].unsqueeze(2).broadcast_to([P, h, co]),
                        op=OP.mult)
                else:
                    nc.vector.tensor_scalar_mul(gat[:], pa[:, :D], rs[:, 0:1])

                bias_b = wsb[f'bias{li}'][:]
                hrow = h_sb[:, bl * 512:(bl + 1) * 512]
                if li < 3:
                    t0 = npool.tile([P, D], F32, name="t0", tag="t0")
                    nc.vector.tensor_tensor(out=t0[:], in0=gat[:], in1=bias_b,
                                            op=OP.add)
                    if li == 0:
                        nc.scalar.activation(out=hrow, in_=t0[:], func=AF.Gelu)
                    else:
                        g1 = npool.tile([P, D], F32, name="g1", tag="g1")
                        nc.scalar.activation(out=g1[:], in_=t0[:], func=AF.Gelu)
                        nc.vector.tensor_tensor(out=hrow, in0=g1[:], in1=hrow,
                                                op=OP.add)
                else:
                    of = npool.tile([P, OUT], F32, name="of", tag="of")
                    nc.vector.tensor_tensor(out=of[:], in0=gat[:], in1=bias_b,
                                            op=OP.add)
                    nc.vector.tensor_tensor(
                        out=of[:], in0=of[:],
                        in1=res_sb[:, bl * OUT:(bl + 1) * OUT], op=OP.add)
                    nc.sync.dma_start(out=out_ext[rows, :], in_=of[:])

            # ---------------- schedule ----------------
            def run_groups(fn):
                for t in range(NB):
                    fn(t)
                    if t == SPLIT - 1:
                        yield 0
                    elif t == NB - 1:
                        yield 1

            # layer 0 node phase with chunked AG
            for j in run_groups(lambda t: node_block(0, t)):
                ag_group(0, j)
            # fused: edge(li-1) + node(li) per block, AGs interleaved
            for li in range(1, 4):
                def fused(t, _li=li):
                    edge_block(_li - 1, t)
                    node_block(_li, t)
                for j in run_groups(fused):
                    ag_group(li, j)
            for bl in range(NB):
                edge_block(3, bl)

    nc.compile()
    return nc


# ------------------------------------------------------------------- kernel

_CACHE = {}


def kernel(**inputs):
    inputs = {k: np.asarray(v) for k, v in inputs.items()}
    new_id, gidx, didx, CA, CB = _build_partition(inputs['edge_index'])
    ws = _fold_weights(inputs)

    x = np.asarray(inputs['x'], np.float32)
    xp = np.zeros((NTOT, IN), np.float32)
    xp[new_id] = x

    key = (CA, CB)
    if key not in _CACHE:
        _CACHE[key] = _build_program(CA, CB)
    nc = _CACHE[key]

    in_maps = []
    for c in range(NCORES):
        m = dict(
            x_s=np.ascontiguousarray(xp[c * NPC:(c + 1) * NPC]),
            gidx=np.ascontiguousarray(gidx[c]),
            didx=np.ascontiguousarray(didx[c]),
        )
        m.update(ws)
        in_maps.append(m)

    res = run_bass_kernel_spmd(nc, in_maps, core_ids=list(range(NCORES)))
    global _LAST_RES
    _LAST_RES = res
    outs = np.concatenate([r["out"] for r in res.results], axis=0)  # [NTOT, 3]
    return np.ascontiguousarray(outs[new_id])


_LAST_RES = None


# revision 22
# speedup vs baseline: 1.2270x; 1.2270x over previous
"""4-layer GAT (GNN message passing) on 8 TRN2 NeuronCores.

Strategy (graph/data parallel per sharding hint):
- Nodes are permuted into 392 degree-balanced blocks of 128 (49 blocks/core).
- Node phase (sharded): LN (folded into weights) -> bf16 projection matmul
  with attention-logit columns folded in -> per-core table shard
  [msg(512) | al_src(8) | pad to 640] bf16, AllGathered to all cores.
- Edge phase (dst-sharded): per dst-block, TWO InstDMAGatherAnt gathers
  (table halves, int16 idx limit) fetch all source rows in 2 instructions
  (SWDGE fixed overhead ~1us/instr makes per-chunk indirect DMA untenable).
  One-hot S built on-device via is_equal; dst logits broadcast to edges via
  transpose(S) matmul; softmax without segment-max (exactly cancels);
  attention-weighted scatter-add via one-hot matmuls into PSUM.
- Final layer output + residual written per-core; host inverse-permutes.
"""
import heapq
import ml_dtypes
import numpy as np

from concourse import bass, bacc, mybir, tile
from concourse.bass_utils import run_bass_kernel_spmd
from concourse.masks import make_identity

F32 = mybir.dt.float32
BF16 = mybir.dt.bfloat16
I16 = mybir.dt.int16
AF = mybir.ActivationFunctionType
AX = mybir.AxisListType
OP = mybir.AluOpType

N = 50000
IN = 128
OUT = 3
NEG_SLOPE = 0.2
EPS = 1e-6
P = 128
NCORES = 8
NB = 49                    # dst blocks per core
NPC = NB * P               # 6272 nodes per core
NTOT = NCORES * NPC        # 50176 padded nodes
PADV = 200.0               # didx value for padding edge slots (no dst match)
# The full table is split into two Shared tensors (A: blocks 0..25, B: blocks
# 26..48 of every core) so the AllGather can be chunked (Shared DRAM tensors
# allow a single writing instruction) and dma_gather's int16 row indices fit.
SPLIT = 26
ROWS_A = SPLIT * NCORES * P          # 26624
ROWS_B = (NB - SPLIT) * NCORES * P   # 23552

# layer configs: (D_in, D_out(heads*co), heads, co, table_cols)
LAYERS = [(IN, 512, 8, 64, 640), (512, 512, 8, 64, 640),
          (512, 512, 8, 64, 640), (512, 3, 1, 3, 128)]


# ----------------------------------------------------------------- host prep

def _build_partition(edge_index):
    src0 = edge_index[0].astype(np.int64)
    dst0 = edge_index[1].astype(np.int64)
    loops = np.arange(N, dtype=np.int64)
    src = np.concatenate([src0, loops])
    dst = np.concatenate([dst0, loops])
    deg = np.bincount(dst, minlength=N)

    nblocks = NCORES * NB
    order = np.argsort(-deg, kind="stable")
    heap = [(0, 0, b) for b in range(nblocks)]
    heapq.heapify(heap)
    blk_of = np.empty(N, np.int64)
    slot_of = np.empty(N, np.int64)
    spill = []
    for v in order:
        while True:
            load, cnt, b = heapq.heappop(heap)
            if cnt < P:
                break
            spill.append((load, cnt, b))
        blk_of[v] = b
        slot_of[v] = cnt
        heapq.heappush(heap, (load + int(deg[v]), cnt + 1, b))
        for item in spill:
            heapq.heappush(heap, item)
        spill.clear()

    new_id = blk_of * P + slot_of
    filled = np.zeros(NTOT, bool)
    filled[new_id] = True
    pad_ids = np.nonzero(~filled)[0]

    # table-row numbering inside the A/B Shared tensors
    bl_all = (new_id // P) % NB
    c_all = new_id // (NB * P)
    slot_all = new_id % P
    in_a = bl_all < SPLIT
    row_tab = np.where(
        in_a,
        (c_all * SPLIT + bl_all) * P + slot_all,
        (c_all * (NB - SPLIT) + (bl_all - SPLIT)) * P + slot_all)
    nsrc = np.where(in_a[src], row_tab[src], row_tab[src] + ROWS_A)
    ndst = new_id[dst]
    # dummy edges so padded dst slots get a nonzero softmax denominator
    nsrc = np.concatenate([nsrc, np.zeros(len(pad_ids), np.int64)])
    ndst = np.concatenate([ndst, pad_ids])

    eblk = ndst // P
    order_e = np.lexsort((ndst, eblk))
    nsrc, ndst, eblk = nsrc[order_e], ndst[order_e], eblk[order_e]
    starts_all = np.searchsorted(eblk, np.arange(nblocks + 1))

    # per-block A/B split (src < HALF vs >=) for int16 dma_gather indices
    blocks = []
    ca_max = cb_max = 0
    for b in range(nblocks):
        lo, hi = starts_all[b], starts_all[b + 1]
        bs = nsrc[lo:hi]
        bd = ndst[lo:hi] - b * P
        isa = bs < ROWS_A
        sa, da = bs[isa], bd[isa]
        sb_, db = bs[~isa] - ROWS_A, bd[~isa]
        ca = max(1, (len(sa) + P - 1) // P)
        cb = max(1, (len(sb_) + P - 1) // P)
        ca_max, cb_max = max(ca_max, ca), max(cb_max, cb)
        blocks.append((sa, da, sb_, db))

    CA, CB = ca_max, cb_max
    NCHB = CA + CB
    gidx = np.zeros((NCORES, P, NB * NCHB * 8), np.int16)
    didx = np.full((NCORES, P, NB * NCHB), PADV, np.float32)

    def wrap16(seq, cnum):
        out = np.zeros(cnum * P, np.int16)
        out[:len(seq)] = seq.astype(np.int16)
        # [16, cnum*8] wrapped, replicated to all 8 Q7-core stripes
        return np.tile(out.reshape(cnum * 8, 16).T, (8, 1))

    def slots(seq, cnum):
        out = np.full(cnum * P, PADV, np.float32)
        out[:len(seq)] = seq.astype(np.float32)
        return out.reshape(cnum, P).T             # [P, cnum]

    for b in range(nblocks):
        c, bl = divmod(b, NB)
        sa, da, sb_, db = blocks[b]
        col = bl * NCHB * 8
        gidx[c, :, col:col + CA * 8] = wrap16(sa, CA)
        gidx[c, :, col + CA * 8:col + NCHB * 8] = wrap16(sb_, CB)
        dcol = bl * NCHB
        didx[c, :, dcol:dcol + CA] = slots(da, CA)
        didx[c, :, dcol + CA:dcol + NCHB] = slots(db, CB)
    return new_id, gidx, didx, CA, CB


def _fold_weights(inputs):
    ws = {}
    for i, (ci, D, h, co, tc) in enumerate(LAYERS):
        g = np.asarray(inputs[f'ln_g{i}'], np.float32)
        b = np.asarray(inputs[f'ln_b{i}'], np.float32)
        W = np.asarray(inputs[f'W{i}'], np.float32)
        asr = np.asarray(inputs[f'asr{i}'], np.float32)       # [h, co]
        adt = np.asarray(inputs[f'adt{i}'], np.float32)
        wp = np.ascontiguousarray((W * g[None, :]).T)         # [ci, D] f32
        cv = (W @ b).astype(np.float32)                       # [D]
        wp3 = wp.reshape(ci, h, co)
        wa = np.concatenate([
            np.einsum('ihc,hc->ih', wp3, asr),
            np.einsum('ihc,hc->ih', wp3, adt)], axis=1)       # [ci, 2h]
        c0 = np.concatenate([
            np.einsum('hc,hc->h', cv.reshape(h, co), asr),
            np.einsum('hc,hc->h', cv.reshape(h, co), adt)])   # [2h]
        ws[f'wp{i}'] = wp.astype(ml_dtypes.bfloat16)
        ws[f'wa{i}'] = np.ascontiguousarray(wa).astype(ml_dtypes.bfloat16)
        ws[f'cv{i}'] = cv.reshape(1, D)
        ws[f'c0{i}'] = c0.reshape(1, 2 * h).astype(np.float32)
        ws[f'bias{i}'] = np.asarray(inputs[f'b{i}'], np.float32).reshape(1, D)
    ws['res_wt'] = np.ascontiguousarray(
        np.asarray(inputs['res_W'], np.float32).T).astype(ml_dtypes.bfloat16)
    ws['res_b'] = np.asarray(inputs['res_b'], np.float32).reshape(1, OUT)
    ws['iota'] = np.arange(P, dtype=np.float32).reshape(1, P)
    return ws


# -------------------------------------------------------------- bass program

_DBG = False


def _dump(nc, pool, dst, src_ap, cols, dtype=F32):
    t = pool.tile([P, cols], dtype, name="dbgt")
    nc.vector.tensor_copy(out=t[:], in_=src_ap)
    nc.sync.dma_start(out=dst[:], in_=t[:])


def _build_program(CA, CB):
    nc = bacc.Bacc("TRN2", target_bir_lowering=False, debug=False,
                   num_devices=NCORES)
    NCHB = CA + CB

    x_s = nc.dram_tensor("x_s", [NPC, IN], F32, kind="ExternalInput")
    gidx_d = nc.dram_tensor("gidx", [P, NB * NCHB * 8], I16, kind="ExternalInput")
    didx_d = nc.dram_tensor("didx", [P, NB * NCHB], F32, kind="ExternalInput")
    wdram = {}
    for i, (ci, D, h, co, tc) in enumerate(LAYERS):
        wdram[f'wp{i}'] = nc.dram_tensor(f"wp{i}", [ci, D], BF16, kind="ExternalInput")
        wdram[f'wa{i}'] = nc.dram_tensor(f"wa{i}", [ci, 2 * h], BF16, kind="ExternalInput")
        wdram[f'cv{i}'] = nc.dram_tensor(f"cv{i}", [1, D], F32, kind="ExternalInput")
        wdram[f'c0{i}'] = nc.dram_tensor(f"c0{i}", [1, 2 * h], F32, kind="ExternalInput")
        wdram[f'bias{i}'] = nc.dram_tensor(f"bias{i}", [1, D], F32, kind="ExternalInput")
    wdram['res_wt'] = nc.dram_tensor("res_wt", [IN, OUT], BF16, kind="ExternalInput")
    wdram['res_b'] = nc.dram_tensor("res_b", [1, OUT], F32, kind="ExternalInput")
    wdram['iota'] = nc.dram_tensor("iota", [1, P], F32, kind="ExternalInput")
    out_ext = nc.dram_tensor("out", [NPC, OUT], F32, kind="ExternalOutput")
    dbg = {}
    if _DBG:
        dbg['shard0'] = nc.dram_tensor("dbg_shard0", [NPC, 640], BF16,
                                       kind="ExternalOutput")

        dbg['g0'] = nc.dram_tensor("dbg_g0", [P, (CA + CB) * 640], BF16,
                                   kind="ExternalOutput")
        dbg['s0'] = nc.dram_tensor("dbg_s0", [P, (CA + CB) * P], BF16,
                                   kind="ExternalOutput")
        dbg['ade0'] = nc.dram_tensor("dbg_ade0", [P, (CA + CB) * 8], F32,
                                     kind="ExternalOutput")
        dbg['pa0'] = nc.dram_tensor("dbg_pa0", [P, 512], F32,
                                    kind="ExternalOutput")
        dbg['pb0'] = nc.dram_tensor("dbg_pb0", [P, 16], F32,
                                    kind="ExternalOutput")
        dbg['h0'] = nc.dram_tensor("dbg_h0", [NPC, 512], F32,
                                   kind="ExternalOutput")

    rg = [list(range(NCORES))]

    with tile.TileContext(nc) as tc_:
        with (
            tc_.tile_pool(name="dram", bufs=1, space="DRAM") as dpool,
            tc_.tile_pool(name="const", bufs=1) as cpool,
            tc_.tile_pool(name="node", bufs=2) as npool,
            tc_.tile_pool(name="stats", bufs=3) as spool,
            tc_.tile_pool(name="edge", bufs=2) as epool,
            tc_.tile_pool(name="big", bufs=2) as bpool,
            tc_.tile_pool(name="psA", bufs=2, space="PSUM") as psA,
            tc_.tile_pool(name="psB", bufs=2, space="PSUM") as psB,
            tc_.tile_pool(name="psT", bufs=2, space="PSUM") as psT,
        ):
            shard, fullA, fullB = {}, {}, {}
            for li, (ci, D, h, co, tcl) in enumerate(LAYERS):
                shard[li] = dpool.tile([NPC, tcl], BF16, name=f"shard{li}")
                fullA[li] = dpool.tile([ROWS_A, tcl], BF16, name=f"fullA{li}",
                                       addr_space="Shared")
                fullB[li] = dpool.tile([ROWS_B, tcl], BF16, name=f"fullB{li}",
                                       addr_space="Shared")

            # --- constants
            ident = cpool.tile([P, P], F32, name="ident")
            make_identity(nc, ident[:])
            ident_bf = cpool.tile([P, P], BF16, name="ident_bf")
            nc.vector.tensor_copy(out=ident_bf[:], in_=ident[:])
            gidx = cpool.tile([P, NB * NCHB * 8], I16, name="gidx")
            nc.sync.dma_start(out=gidx[:], in_=gidx_d[:])
            didx = cpool.tile([P, NB * NCHB], F32, name="didx")
            nc.sync.dma_start(out=didx[:], in_=didx_d[:])
            iota_f = cpool.tile([P, P], F32, name="iota_f")
            nc.sync.dma_start(out=iota_f[:], in_=wdram['iota'][:].to_broadcast([P, P]))
            wsb = {}
            for i, (ci, D, h, co, tcl) in enumerate(LAYERS):
                kc = ci // P
                wt = cpool.tile([P, kc * D], BF16, name=f"wsb{i}")
                nc.sync.dma_start(
                    out=wt[:].rearrange("p (k d) -> p k d", k=kc),
                    in_=wdram[f'wp{i}'][:].rearrange("(k p) d -> p k d", p=P))
                wsb[f'wp{i}'] = wt
                wat = cpool.tile([P, kc * 2 * h], BF16, name=f"wa{i}sb")
                nc.sync.dma_start(
                    out=wat[:].rearrange("p (k d) -> p k d", k=kc),
                    in_=wdram[f'wa{i}'][:].rearrange("(k p) d -> p k d", p=P))
                wsb[f'wa{i}'] = wat
                for nm, w in (("cv", D), ("c0", 2 * h), ("bias", D)):
                    vt = cpool.tile([P, w], F32, name=f"{nm}{i}sb")
                    nc.sync.dma_start(
                        out=vt[:], in_=wdram[f'{nm}{i}'][:].to_broadcast([P, w]))
                    wsb[f'{nm}{i}'] = vt
            res_wt = cpool.tile([IN, OUT], BF16, name="res_wt_sb")
            nc.sync.dma_start(out=res_wt[:], in_=wdram['res_wt'][:])
            res_b = cpool.tile([P, OUT], F32, name="res_b_sb")
            nc.sync.dma_start(out=res_b[:],
                              in_=wdram['res_b'][:].to_broadcast([P, OUT]))
            res_sb = cpool.tile([P, NB * OUT], F32, name="res_sb")
            adst_sb = cpool.tile([P, NB * 8], BF16, name="adst_sb")
            h_sb = cpool.tile([P, NB * 512], BF16, name="h_sb")
            eps_t = cpool.tile([P, 1], F32, name="eps_t")
            nc.gpsimd.memset(eps_t[:], EPS)

            # ---------------- per-block node compute ----------------
            def node_block(li, t):
                ci, D, h, co, tcl = LAYERS[li]
                kc = ci // P
                rows = slice(t * P, (t + 1) * P)
                if li == 0:
                    ht = npool.tile([P, ci], F32, name="ht", tag="ht")
                    nc.sync.dma_start(out=ht[:], in_=x_s[rows, :])
                    xt_ps = psT.tile([P, P], BF16, name="xt_ps", tag="tp")
                    zx = npool.tile([P, P], BF16, name="zx", tag="zx")
                    nc.vector.tensor_copy(out=zx[:], in_=ht[:])
                    nc.tensor.transpose(out=xt_ps[:], in_=zx[:],
                                        identity=ident_bf[:])
                    xt_sb = npool.tile([P, P], BF16, name="xt_sb", tag="xt")
                    nc.vector.tensor_copy(out=xt_sb[:], in_=xt_ps[:])
                    res_ps = psB.tile([P, 16], F32, name="res_ps", tag="mmB")
                    nc.tensor.matmul(out=res_ps[:, :OUT], lhsT=xt_sb[:],
                                     rhs=res_wt[:], start=True, stop=True)
                    nc.vector.tensor_tensor(
                        out=res_sb[:, t * OUT:(t + 1) * OUT],
                        in0=res_ps[:, :OUT], in1=res_b[:], op=OP.add)
                else:
                    ht = None

                hin = ht[:] if li == 0 else h_sb[:, t * 512:t * 512 + ci]
                # layernorm (gamma/beta folded into wp/cv)
                mu_n = spool.tile([P, 1], F32, name="mu_n", tag="st1")
                nc.vector.reduce_sum(out=mu_n[:], in_=hin, axis=AX.X)
                nc.vector.tensor_scalar_mul(mu_n[:], mu_n[:], -1.0 / ci)
                sq = npool.tile([P, ci], F32, name="sq", tag="sq")
                ssq = spool.tile([P, 1], F32, name="ssq", tag="st2")
                nc.scalar.activation(out=sq[:], in_=hin, func=AF.Square,
                                     bias=mu_n[:, :1], accum_out=ssq[:, :1])
                std = spool.tile([P, 1], F32, name="std", tag="st3")
                nc.scalar.activation(out=std[:], in_=ssq[:], func=AF.Sqrt,
                                     scale=1.0 / ci, bias=eps_t[:, :1])
                rstd = spool.tile([P, 1], F32, name="rstd", tag="st4")
                nc.vector.reciprocal(out=rstd[:], in_=std[:])
                mur = spool.tile([P, 1], F32, name="mur", tag="st5")
                nc.vector.tensor_tensor(out=mur[:], in0=mu_n[:], in1=rstd[:],
                                        op=OP.mult)
                z = npool.tile([P, ci], BF16, name="z", tag="z")
                nc.scalar.activation(out=z[:], in_=hin, func=AF.Identity,
                                     scale=rstd[:, :1], bias=mur[:, :1])

                # projection (+ folded logit columns)
                pp = psA.tile([P, 512], F32, name="pp", tag="mmA")
                pl = psB.tile([P, 16], F32, name="pl", tag="mmB")
                wt = wsb[f'wp{li}']
                wat = wsb[f'wa{li}']
                for k in range(kc):
                    zt_ps = psT.tile([P, P], BF16, name="zt_ps", tag="tp")
                    nc.tensor.transpose(out=zt_ps[:], in_=z[:, k * P:(k + 1) * P],
                                        identity=ident_bf[:])
                    zt_sb = npool.tile([P, P], BF16, name="zt_sb", tag="zt")
                    nc.vector.tensor_copy(out=zt_sb[:], in_=zt_ps[:])
                    nc.tensor.matmul(out=pp[:, :D], lhsT=zt_sb[:],
                                     rhs=wt[:, k * D:(k + 1) * D],
                                     start=(k == 0), stop=(k == kc - 1))
                    nc.tensor.matmul(out=pl[:, :2 * h], lhsT=zt_sb[:],
                                     rhs=wat[:, k * 2 * h:(k + 1) * 2 * h],
                                     start=(k == 0), stop=(k == kc - 1))

                ptile = npool.tile([P, tcl], BF16, name="ptile", tag="ptb")
                nc.vector.tensor_tensor(
                    out=ptile[:, 0:D], in0=pp[:, :D],
                    in1=wsb[f'cv{li}'][:], op=OP.add)
                lsum = npool.tile([P, 2 * h], F32, name="lsum", tag="lsum")
                nc.vector.tensor_tensor(
                    out=lsum[:], in0=pl[:, :2 * h],
                    in1=wsb[f'c0{li}'][:], op=OP.add)
                nc.scalar.copy(out=ptile[:, D:D + h], in_=lsum[:, 0:h])
                nc.scalar.copy(out=adst_sb[:, t * 8:t * 8 + h],
                               in_=lsum[:, h:2 * h])
                nc.sync.dma_start(out=shard[li][rows, :], in_=ptile[:])

            def ag_group(li, j):
                if j == 0:
                    rows = slice(0, SPLIT * P)
                    ftab = fullA[li]
                else:
                    rows = slice(SPLIT * P, NB * P)
                    ftab = fullB[li]
                nc.gpsimd.collective_compute(
                    "AllGather", OP.bypass, replica_groups=rg,
                    ins=[shard[li][rows, :].opt()], outs=[ftab[:].opt()])

            # ---------------- per-block edge compute ----------------
            def edge_block(li, bl):
                ci, D, h, co, tcl = LAYERS[li]
                rows = slice(bl * P, (bl + 1) * P)
                g = bpool.tile([P, NCHB * tcl], BF16, name="g", tag="g")
                col = bl * NCHB * 8
                nc.gpsimd.dma_gather(
                    out_ap=g[:, 0:CA * tcl].rearrange("p (c t) -> p c t", c=CA),
                    in_ap=fullA[li][:],
                    idxs_ap=gidx[:, col:col + CA * 8],
                    num_idxs=CA * P, num_idxs_reg=CA * P, elem_size=tcl)
                nc.gpsimd.dma_gather(
                    out_ap=g[:, CA * tcl:].rearrange("p (c t) -> p c t", c=CB),
                    in_ap=fullB[li][:],
                    idxs_ap=gidx[:, col + CA * 8:col + NCHB * 8],
                    num_idxs=CB * P, num_idxs_reg=CB * P, elem_size=tcl)
                gv = g[:].rearrange("p (c t) -> p c t", c=NCHB)

                # one-hot S for all chunks (on-device, from didx)
                dcol = bl * NCHB
                s_all = epool.tile([P, NCHB * P], BF16, name="s_all", tag="S")
                sv = s_all[:].rearrange("p (c d) -> p c d", c=NCHB)
                nc.vector.tensor_tensor(
                    out=sv,
                    in0=didx[:, dcol:dcol + NCHB].unsqueeze(2)
                        .broadcast_to([P, NCHB, P]),
                    in1=iota_f[:].unsqueeze(1).broadcast_to([P, NCHB, P]),
                    op=OP.is_equal)

                # dst logits broadcast to edges: ade = S^T-matmul
                ade = psB.tile([P, NCHB * 8], F32, name="ade", tag="adps")
                for ch in range(NCHB):
                    t_ps = psT.tile([P, P], BF16, name="t_ps", tag="tp")
                    nc.tensor.transpose(out=t_ps[:],
                                        in_=s_all[:, ch * P:(ch + 1) * P],
                                        identity=ident_bf[:])
                    t_sb = epool.tile([P, P], BF16, name="t_sb", tag="T")
                    if ch % 2 == 0:
                        nc.scalar.copy(out=t_sb[:], in_=t_ps[:])
                    else:
                        nc.vector.tensor_copy(out=t_sb[:], in_=t_ps[:])
                    nc.tensor.matmul(
                        out=ade[:, ch * 8:ch * 8 + h], lhsT=t_sb[:],
                        rhs=adst_sb[:, bl * 8:bl * 8 + h],
                        start=True, stop=True)

                if _DBG and li == 0 and bl == 0:
                    nc.sync.dma_start(out=dbg['g0'][:], in_=g[:])
                    nc.sync.dma_start(out=dbg['s0'][:], in_=s_all[:])
                    _dump(nc, npool, dbg['ade0'], ade[:], NCHB * 8)

                # e = prelu(al_src + al_dst); exp(e) -> g logit slot
                eraw = epool.tile([P, NCHB * h], F32, name="eraw", tag="eraw")
                nc.vector.tensor_tensor(
                    out=eraw[:].rearrange("p (c h) -> p c h", c=NCHB),
                    in0=gv[:, :, D:D + h],
                    in1=ade[:].rearrange("p (c h) -> p c h", c=NCHB)[:, :, 0:h],
                    op=OP.add)
                el = epool.tile([P, NCHB * h], F32, name="el", tag="el")
                nc.scalar.activation(out=el[:], in_=eraw[:], func=AF.Prelu,
                                     alpha=NEG_SLOPE)
                nc.scalar.activation(
                    out=gv[:, :, D:D + h],
                    in_=el[:].rearrange("p (c h) -> p c h", c=NCHB),
                    func=AF.Exp)

                # weight messages in place per chunk
                for ch in range(NCHB):
                    ms = g[:, ch * tcl:ch * tcl + D]
                    nc.vector.tensor_tensor(
                        out=ms.rearrange("p (h c) -> p h c", h=h),
                        in0=ms.rearrange("p (h c) -> p h c", h=h),
                        in1=g[:, ch * tcl + D:ch * tcl + D + h]
                            .unsqueeze(2).broadcast_to([P, h, co]),
                        op=OP.mult)

                # scatter-add via one-hot matmuls
                pa = psA.tile([P, 512], F32, name="pa", tag="mmA")
                pb = psB.tile([P, 16], F32, name="pb", tag="mmB")
                for ch in range(NCHB):
                    nc.tensor.matmul(
                        out=pa[:, :D], lhsT=s_all[:, ch * P:(ch + 1) * P],
                        rhs=g[:, ch * tcl:ch * tcl + D],
                        start=(ch == 0), stop=(ch == NCHB - 1))
                    nc.tensor.matmul(
                        out=pb[:, :h], lhsT=s_all[:, ch * P:(ch + 1) * P],
                        rhs=g[:, ch * tcl + D:ch * tcl + D + h],
                        start=(ch == 0), stop=(ch == NCHB - 1))
                if _DBG and li == 0 and bl == 0:
                    _dump(nc, npool, dbg['pa0'], pa[:, :512], 512)
                    _dump(nc, npool, dbg['pb0'], pb[:, :16], 16)
                rs = spool.tile([P, h], F32, name="rs", tag="rs")
                nc.vector.reciprocal(out=rs[:], in_=pb[:, :h])
                gat = npool.tile([P, D], F32, name="gat", tag="gat")
                if h > 1:
                    nc.vector.tensor_tensor(
                        out=gat[:].rearrange("p (h c) -> p h c", h=h),
                        in0=pa[:, :D].rearrange("p (h c) -> p h c", h=h),
                        in1=rs[:].unsqueeze(2).broadcast_to([P, h, co]),
                        op=OP.mult)
                else:
                    nc.vector.tensor_scalar_mul(gat[:], pa[:, :D], rs[:, 0:1])

                bias_b = wsb[f'bias{li}'][:]
                hrow = h_sb[:, bl * 512:(bl + 1) * 512]
                if li < 3:
                    t0 = npool.tile([P, D], F32, name="t0", tag="t0")
                    nc.vector.tensor_tensor(out=t0[:], in0=gat[:], in1=bias_b,
                                            op=OP.add)
                    if li == 0:
                        nc.scalar.activation(out=hrow, in_=t0[:], func=AF.Gelu)
                    else:
                        g1 = npool.tile([P, D], F32, name="g1", tag="g1")
                        nc.scalar.activation(out=g1[:], in_=t0[:], func=AF.Gelu)
                        nc.vector.tensor_tensor(out=hrow, in0=g1[:], in1=hrow,
                                                op=OP.add)
                else:
                    of = npool.tile([P, OUT], F32, name="of", tag="of")
                    nc.vector.tensor_tensor(out=of[:], in0=gat[:], in1=bias_b,
                                            op=OP.add)
                    nc.vector.tensor_tensor(
                        out=of[:], in0=of[:],
                        in1=res_sb[:, bl * OUT:(bl + 1) * OUT], op=OP.add)
                    nc.sync.dma_start(out=out_ext[rows, :], in_=of[:])

            # ---------------- schedule ----------------
            def run_groups(fn):
                for t in range(NB):
                    fn(t)
                    if t == SPLIT - 1:
                        yield 0
                    elif t == NB - 1:
                        yield 1

            for li in range(4):
                for j in run_groups(lambda t, _li=li: node_block(_li, t)):
                    ag_group(li, j)
                for bl in range(NB):
                    edge_block(li, bl)

    nc.compile()
    return nc


# ------------------------------------------------------------------- kernel

_CACHE = {}


def kernel(**inputs):
    inputs = {k: np.asarray(v) for k, v in inputs.items()}
    new_id, gidx, didx, CA, CB = _build_partition(inputs['edge_index'])
    ws = _fold_weights(inputs)

    x = np.asarray(inputs['x'], np.float32)
    xp = np.zeros((NTOT, IN), np.float32)
    xp[new_id] = x

    key = (CA, CB)
    if key not in _CACHE:
        _CACHE[key] = _build_program(CA, CB)
    nc = _CACHE[key]

    in_maps = []
    for c in range(NCORES):
        m = dict(
            x_s=np.ascontiguousarray(xp[c * NPC:(c + 1) * NPC]),
            gidx=np.ascontiguousarray(gidx[c]),
            didx=np.ascontiguousarray(didx[c]),
        )
        m.update(ws)
        in_maps.append(m)

    res = run_bass_kernel_spmd(nc, in_maps, core_ids=list(range(NCORES)))
    global _LAST_RES
    _LAST_RES = res
    outs = np.concatenate([r["out"] for r in res.results], axis=0)  # [NTOT, 3]
    return np.ascontiguousarray(outs[new_id])


_LAST_RES = None


# revision 27
# speedup vs baseline: 1.3291x; 1.0832x over previous
"""4-layer GAT (GNN message passing) on 8 TRN2 NeuronCores.

Strategy (graph/data parallel per sharding hint):
- Nodes are permuted into 392 degree-balanced blocks of 128 (49 blocks/core).
- Node phase (sharded): LN (folded into weights) -> bf16 projection matmul
  with attention-logit columns folded in -> per-core table shard
  [msg(512) | al_src(8) | pad to 640] bf16, AllGathered to all cores.
- Edge phase (dst-sharded): per dst-block, TWO InstDMAGatherAnt gathers
  (table halves, int16 idx limit) fetch all source rows in 2 instructions
  (SWDGE fixed overhead ~1us/instr makes per-chunk indirect DMA untenable).
  One-hot S built on-device via is_equal; dst logits broadcast to edges via
  transpose(S) matmul; softmax without segment-max (exactly cancels);
  attention-weighted scatter-add via one-hot matmuls into PSUM.
- Final layer output + residual written per-core; host inverse-permutes.
"""
import heapq
import ml_dtypes
import numpy as np

from concourse import bass, bacc, mybir, tile
from concourse.bass_utils import run_bass_kernel_spmd
from concourse.masks import make_identity

F32 = mybir.dt.float32
BF16 = mybir.dt.bfloat16
I16 = mybir.dt.int16
AF = mybir.ActivationFunctionType
AX = mybir.AxisListType
OP = mybir.AluOpType

N = 50000
IN = 128
OUT = 3
NEG_SLOPE = 0.2
EPS = 1e-6
P = 128
NCORES = 8
NB = 49                    # dst blocks per core
NPC = NB * P               # 6272 nodes per core
NTOT = NCORES * NPC        # 50176 padded nodes
PADV = 200.0               # didx value for padding edge slots (no dst match)
# The full table is split into two Shared tensors (A: blocks 0..25, B: blocks
# 26..48 of every core) so the AllGather can be chunked (Shared DRAM tensors
# allow a single writing instruction) and dma_gather's int16 row indices fit.
SPLIT = 24
ROWS_A = SPLIT * NCORES * P          # 24576
ROWS_B = (NB - SPLIT) * NCORES * P   # 25600

# layer configs: (D_in, D_out(heads*co), heads, co, table_cols)
LAYERS = [(IN, 512, 8, 64, 640), (512, 512, 8, 64, 640),
          (512, 512, 8, 64, 640), (512, 3, 1, 3, 128)]


# ----------------------------------------------------------------- host prep

def _build_partition(edge_index):
    # self-loops are handled analytically on-device (the self message is
    # local); only real cross-node edges are gathered
    src = edge_index[0].astype(np.int64)
    dst = edge_index[1].astype(np.int64)
    keep = src != dst
    src, dst = src[keep], dst[keep]
    deg = np.bincount(dst, minlength=N)

    nblocks = NCORES * NB

    def assign(nodes, blocks_list, dA, dB):
        """Greedy bi-criteria: place `nodes` (desc by dA+dB) into blocks_list,
        128 per block, minimizing max(loadA, loadB)."""
        order = nodes[np.argsort(-(dA[nodes] + dB[nodes]), kind="stable")]
        heap = [(0.0, 0, b) for b in blocks_list]
        heapq.heapify(heap)
        loadA = {b: 0.0 for b in blocks_list}
        loadB = {b: 0.0 for b in blocks_list}
        cnt = {b: 0 for b in blocks_list}
        blk = {}
        slot = {}
        for v in order:
            va, vb = dA[v], dB[v]
            while True:
                key, _, b = heapq.heappop(heap)
                if cnt[b] >= P:
                    continue
                nk = max(loadA[b] + va, loadB[b] + vb)
                if nk > key + 1e-9:
                    heapq.heappush(heap, (nk, cnt[b], b))
                    continue
                break
            blk[v] = b
            slot[v] = cnt[b]
            loadA[b] += va
            loadB[b] += vb
            cnt[b] += 1
            if cnt[b] < P:
                heapq.heappush(heap, (max(loadA[b], loadB[b]), cnt[b], b))
        return blk, slot

    # pass 1: balance total in-degree to fix each node's SIDE
    # (side A iff block index bl = blk % NB < SPLIT)
    dT = deg.astype(np.float64)
    allb = list(range(nblocks))
    blk1, slot1 = assign(np.arange(N), allb, dT, np.zeros(N))
    blk_of = np.empty(N, np.int64)
    for v, b in blk1.items():
        blk_of[v] = b
    # sides now fixed; per-node (dA, dB) are exact and invariant under
    # within-side moves
    src_is_a = (blk_of[src] % NB) < SPLIT
    dA = np.bincount(dst[src_is_a], minlength=N).astype(np.float64)
    dB = np.bincount(dst[~src_is_a], minlength=N).astype(np.float64)
    # pass 2: rebalance each side independently (sides cannot change)
    a_nodes = np.nonzero((blk_of % NB) < SPLIT)[0]
    b_nodes = np.nonzero((blk_of % NB) >= SPLIT)[0]
    a_blocks = [b for b in allb if b % NB < SPLIT]
    b_blocks = [b for b in allb if b % NB >= SPLIT]
    blkA, slotA = assign(a_nodes, a_blocks, dA, dB)
    blkB, slotB = assign(b_nodes, b_blocks, dA, dB)
    slot_of = np.empty(N, np.int64)
    for v in a_nodes:
        blk_of[v] = blkA[v]
        slot_of[v] = slotA[v]
    for v in b_nodes:
        blk_of[v] = blkB[v]
        slot_of[v] = slotB[v]

    new_id = blk_of * P + slot_of

    # table-row numbering inside the A/B Shared tensors
    bl_all = (new_id // P) % NB
    c_all = new_id // (NB * P)
    slot_all = new_id % P
    in_a = bl_all < SPLIT
    row_tab = np.where(
        in_a,
        (c_all * SPLIT + bl_all) * P + slot_all,
        (c_all * (NB - SPLIT) + (bl_all - SPLIT)) * P + slot_all)
    nsrc = np.where(in_a[src], row_tab[src], row_tab[src] + ROWS_A)
    ndst = new_id[dst]

    eblk = ndst // P
    order_e = np.lexsort((ndst, eblk))
    nsrc, ndst, eblk = nsrc[order_e], ndst[order_e], eblk[order_e]
    starts_all = np.searchsorted(eblk, np.arange(nblocks + 1))

    # per-block A/B split (src < HALF vs >=) for int16 dma_gather indices
    blocks = []
    ca_max = cb_max = 0
    for b in range(nblocks):
        lo, hi = starts_all[b], starts_all[b + 1]
        bs = nsrc[lo:hi]
        bd = ndst[lo:hi] - b * P
        isa = bs < ROWS_A
        sa, da = bs[isa], bd[isa]
        sb_, db = bs[~isa] - ROWS_A, bd[~isa]
        ca = max(1, (len(sa) + P - 1) // P)
        cb = max(1, (len(sb_) + P - 1) // P)
        ca_max, cb_max = max(ca_max, ca), max(cb_max, cb)
        blocks.append((sa, da, sb_, db))

    CA, CB = ca_max, cb_max
    NCHB = CA + CB
    gidx = np.zeros((NCORES, P, NB * NCHB * 8), np.int16)
    didx = np.full((NCORES, P, NB * NCHB), PADV, np.float32)

    def wrap16(seq, cnum):
        out = np.zeros(cnum * P, np.int16)
        out[:len(seq)] = seq.astype(np.int16)
        # [16, cnum*8] wrapped, replicated to all 8 Q7-core stripes
        return np.tile(out.reshape(cnum * 8, 16).T, (8, 1))

    def slots(seq, cnum):
        out = np.full(cnum * P, PADV, np.float32)
        out[:len(seq)] = seq.astype(np.float32)
        return out.reshape(cnum, P).T             # [P, cnum]

    for b in range(nblocks):
        c, bl = divmod(b, NB)
        sa, da, sb_, db = blocks[b]
        col = bl * NCHB * 8
        gidx[c, :, col:col + CA * 8] = wrap16(sa, CA)
        gidx[c, :, col + CA * 8:col + NCHB * 8] = wrap16(sb_, CB)
        dcol = bl * NCHB
        didx[c, :, dcol:dcol + CA] = slots(da, CA)
        didx[c, :, dcol + CA:dcol + NCHB] = slots(db, CB)
    return new_id, gidx, didx, CA, CB


def _fold_weights(inputs):
    ws = {}
    for i, (ci, D, h, co, tc) in enumerate(LAYERS):
        g = np.asarray(inputs[f'ln_g{i}'], np.float32)
        b = np.asarray(inputs[f'ln_b{i}'], np.float32)
        W = np.asarray(inputs[f'W{i}'], np.float32)
        asr = np.asarray(inputs[f'asr{i}'], np.float32)       # [h, co]
        adt = np.asarray(inputs[f'adt{i}'], np.float32)
        wp = np.ascontiguousarray((W * g[None, :]).T)         # [ci, D] f32
        cv = (W @ b).astype(np.float32)                       # [D]
        wp3 = wp.reshape(ci, h, co)
        wa = np.concatenate([
            np.einsum('ihc,hc->ih', wp3, asr),
            np.einsum('ihc,hc->ih', wp3, adt)], axis=1)       # [ci, 2h]
        c0 = np.concatenate([
            np.einsum('hc,hc->h', cv.reshape(h, co), asr),
            np.einsum('hc,hc->h', cv.reshape(h, co), adt)])   # [2h]
        ws[f'wp{i}'] = wp.astype(ml_dtypes.bfloat16)
        ws[f'wa{i}'] = np.ascontiguousarray(wa).astype(ml_dtypes.bfloat16)
        ws[f'cv{i}'] = cv.reshape(1, D)
        ws[f'c0{i}'] = c0.reshape(1, 2 * h).astype(np.float32)
        ws[f'bias{i}'] = np.asarray(inputs[f'b{i}'], np.float32).reshape(1, D)
    ws['res_wt'] = np.ascontiguousarray(
        np.asarray(inputs['res_W'], np.float32).T).astype(ml_dtypes.bfloat16)
    ws['res_b'] = np.asarray(inputs['res_b'], np.float32).reshape(1, OUT)
    ws['iota'] = np.arange(P, dtype=np.float32).reshape(1, P)
    return ws


# -------------------------------------------------------------- bass program

_DBG = False


def _dump(nc, pool, dst, src_ap, cols, dtype=F32):
    t = pool.tile([P, cols], dtype, name="dbgt")
    nc.vector.tensor_copy(out=t[:], in_=src_ap)
    nc.sync.dma_start(out=dst[:], in_=t[:])


def _build_program(CA, CB):
    nc = bacc.Bacc("TRN2", target_bir_lowering=False, debug=False,
                   num_devices=NCORES)
    NCHB = CA + CB

    x_s = nc.dram_tensor("x_s", [NPC, IN], F32, kind="ExternalInput")
    gidx_d = nc.dram_tensor("gidx", [P, NB * NCHB * 8], I16, kind="ExternalInput")
    didx_d = nc.dram_tensor("didx", [P, NB * NCHB], F32, kind="ExternalInput")
    wdram = {}
    for i, (ci, D, h, co, tc) in enumerate(LAYERS):
        wdram[f'wp{i}'] = nc.dram_tensor(f"wp{i}", [ci, D], BF16, kind="ExternalInput")
        wdram[f'wa{i}'] = nc.dram_tensor(f"wa{i}", [ci, 2 * h], BF16, kind="ExternalInput")
        wdram[f'cv{i}'] = nc.dram_tensor(f"cv{i}", [1, D], F32, kind="ExternalInput")
        wdram[f'c0{i}'] = nc.dram_tensor(f"c0{i}", [1, 2 * h], F32, kind="ExternalInput")
        wdram[f'bias{i}'] = nc.dram_tensor(f"bias{i}", [1, D], F32, kind="ExternalInput")
    wdram['res_wt'] = nc.dram_tensor("res_wt", [IN, OUT], BF16, kind="ExternalInput")
    wdram['res_b'] = nc.dram_tensor("res_b", [1, OUT], F32, kind="ExternalInput")
    wdram['iota'] = nc.dram_tensor("iota", [1, P], F32, kind="ExternalInput")
    out_ext = nc.dram_tensor("out", [NPC, OUT], F32, kind="ExternalOutput")
    dbg = {}
    if _DBG:
        dbg['shard0'] = nc.dram_tensor("dbg_shard0", [NPC, 640], BF16,
                                       kind="ExternalOutput")

        dbg['g0'] = nc.dram_tensor("dbg_g0", [P, (CA + CB) * 640], BF16,
                                   kind="ExternalOutput")
        dbg['s0'] = nc.dram_tensor("dbg_s0", [P, (CA + CB) * P], BF16,
                                   kind="ExternalOutput")
        dbg['ade0'] = nc.dram_tensor("dbg_ade0", [P, (CA + CB) * 8], F32,
                                     kind="ExternalOutput")
        dbg['pa0'] = nc.dram_tensor("dbg_pa0", [P, 512], F32,
                                    kind="ExternalOutput")
        dbg['pb0'] = nc.dram_tensor("dbg_pb0", [P, 16], F32,
                                    kind="ExternalOutput")
        dbg['h0'] = nc.dram_tensor("dbg_h0", [NPC, 512], F32,
                                   kind="ExternalOutput")

    rg = [list(range(NCORES))]

    with tile.TileContext(nc) as tc_:
        with (
            tc_.tile_pool(name="dram", bufs=1, space="DRAM") as dpool,
            tc_.tile_pool(name="const", bufs=1) as cpool,
            tc_.tile_pool(name="node", bufs=2) as npool,
            tc_.tile_pool(name="stats", bufs=3) as spool,
            tc_.tile_pool(name="edge", bufs=2) as epool,
            tc_.tile_pool(name="big", bufs=2) as bpool,
            tc_.tile_pool(name="psA", bufs=2, space="PSUM") as psA,
            tc_.tile_pool(name="psB", bufs=2, space="PSUM") as psB,
            tc_.tile_pool(name="psT", bufs=2, space="PSUM") as psT,
        ):
            shard, fullA, fullB = {}, {}, {}
            for li, (ci, D, h, co, tcl) in enumerate(LAYERS):
                shard[li] = dpool.tile([NPC, tcl], BF16, name=f"shard{li}")
                fullA[li] = dpool.tile([ROWS_A, tcl], BF16, name=f"fullA{li}",
                                       addr_space="Shared")
                fullB[li] = dpool.tile([ROWS_B, tcl], BF16, name=f"fullB{li}",
                                       addr_space="Shared")

            # --- constants
            ident = cpool.tile([P, P], F32, name="ident")
            make_identity(nc, ident[:])
            ident_bf = cpool.tile([P, P], BF16, name="ident_bf")
            nc.vector.tensor_copy(out=ident_bf[:], in_=ident[:])
            gidx = cpool.tile([P, NB * NCHB * 8], I16, name="gidx")
            nc.sync.dma_start(out=gidx[:], in_=gidx_d[:])
            didx = cpool.tile([P, NB * NCHB], F32, name="didx")
            nc.sync.dma_start(out=didx[:], in_=didx_d[:])
            iota_f = cpool.tile([P, P], F32, name="iota_f")
            nc.sync.dma_start(out=iota_f[:], in_=wdram['iota'][:].to_broadcast([P, P]))
            wsb = {}
            for i, (ci, D, h, co, tcl) in enumerate(LAYERS):
                kc = ci // P
                wt = cpool.tile([P, kc * D], BF16, name=f"wsb{i}")
                nc.sync.dma_start(
                    out=wt[:].rearrange("p (k d) -> p k d", k=kc),
                    in_=wdram[f'wp{i}'][:].rearrange("(k p) d -> p k d", p=P))
                wsb[f'wp{i}'] = wt
                wat = cpool.tile([P, kc * 2 * h], BF16, name=f"wa{i}sb")
                nc.sync.dma_start(
                    out=wat[:].rearrange("p (k d) -> p k d", k=kc),
                    in_=wdram[f'wa{i}'][:].rearrange("(k p) d -> p k d", p=P))
                wsb[f'wa{i}'] = wat
                for nm, w in (("cv", D), ("c0", 2 * h), ("bias", D)):
                    vt = cpool.tile([P, w], F32, name=f"{nm}{i}sb")
                    nc.sync.dma_start(
                        out=vt[:], in_=wdram[f'{nm}{i}'][:].to_broadcast([P, w]))
                    wsb[f'{nm}{i}'] = vt
            res_wt = cpool.tile([IN, OUT], BF16, name="res_wt_sb")
            nc.sync.dma_start(out=res_wt[:], in_=wdram['res_wt'][:])
            res_b = cpool.tile([P, OUT], F32, name="res_b_sb")
            nc.sync.dma_start(out=res_b[:],
                              in_=wdram['res_b'][:].to_broadcast([P, OUT]))
            res_sb = cpool.tile([P, NB * OUT], F32, name="res_sb")
            adst_sb = cpool.tile([P, NB * 8], BF16, name="adst_sb")
            h_sb = cpool.tile([P, NB * 512], BF16, name="h_sb")
            eps_t = cpool.tile([P, 1], F32, name="eps_t")
            nc.gpsimd.memset(eps_t[:], EPS)

            # ---------------- per-block node compute ----------------
            def node_block(li, t):
                ci, D, h, co, tcl = LAYERS[li]
                kc = ci // P
                rows = slice(t * P, (t + 1) * P)
                if li == 0:
                    ht = npool.tile([P, ci], F32, name="ht", tag="ht")
                    nc.sync.dma_start(out=ht[:], in_=x_s[rows, :])
                    xt_ps = psT.tile([P, P], BF16, name="xt_ps", tag="tp")
                    zx = npool.tile([P, P], BF16, name="zx", tag="zx")
                    nc.vector.tensor_copy(out=zx[:], in_=ht[:])
                    nc.tensor.transpose(out=xt_ps[:], in_=zx[:],
                                        identity=ident_bf[:])
                    xt_sb = npool.tile([P, P], BF16, name="xt_sb", tag="xt")
                    nc.vector.tensor_copy(out=xt_sb[:], in_=xt_ps[:])
                    res_ps = psB.tile([P, 16], F32, name="res_ps", tag="mmB")
                    nc.tensor.matmul(out=res_ps[:, :OUT], lhsT=xt_sb[:],
                                     rhs=res_wt[:], start=True, stop=True)
                    nc.vector.tensor_tensor(
                        out=res_sb[:, t * OUT:(t + 1) * OUT],
                        in0=res_ps[:, :OUT], in1=res_b[:], op=OP.add)
                else:
                    ht = None

                hin = ht[:] if li == 0 else h_sb[:, t * 512:t * 512 + ci]
                # layernorm (gamma/beta folded into wp/cv)
                mu_n = spool.tile([P, 1], F32, name="mu_n", tag="st1")
                nc.vector.reduce_sum(out=mu_n[:], in_=hin, axis=AX.X)
                nc.vector.tensor_scalar_mul(mu_n[:], mu_n[:], -1.0 / ci)
                sq = npool.tile([P, ci], F32, name="sq", tag="sq")
                ssq = spool.tile([P, 1], F32, name="ssq", tag="st2")
                nc.scalar.activation(out=sq[:], in_=hin, func=AF.Square,
                                     bias=mu_n[:, :1], accum_out=ssq[:, :1])
                std = spool.tile([P, 1], F32, name="std", tag="st3")
                nc.scalar.activation(out=std[:], in_=ssq[:], func=AF.Sqrt,
                                     scale=1.0 / ci, bias=eps_t[:, :1])
                rstd = spool.tile([P, 1], F32, name="rstd", tag="st4")
                nc.vector.reciprocal(out=rstd[:], in_=std[:])
                mur = spool.tile([P, 1], F32, name="mur", tag="st5")
                nc.vector.tensor_tensor(out=mur[:], in0=mu_n[:], in1=rstd[:],
                                        op=OP.mult)
                z = npool.tile([P, ci], BF16, name="z", tag="z")
                nc.scalar.activation(out=z[:], in_=hin, func=AF.Identity,
                                     scale=rstd[:, :1], bias=mur[:, :1])

                # projection (+ folded logit columns)
                pp = psA.tile([P, 512], F32, name="pp", tag="mmA")
                pl = psB.tile([P, 16], F32, name="pl", tag="mmB")
                wt = wsb[f'wp{li}']
                wat = wsb[f'wa{li}']
                for k in range(kc):
                    zt_ps = psT.tile([P, P], BF16, name="zt_ps", tag="tp")
                    nc.tensor.transpose(out=zt_ps[:], in_=z[:, k * P:(k + 1) * P],
                                        identity=ident_bf[:])
                    zt_sb = npool.tile([P, P], BF16, name="zt_sb", tag="zt")
                    nc.vector.tensor_copy(out=zt_sb[:], in_=zt_ps[:])
                    nc.tensor.matmul(out=pp[:, :D], lhsT=zt_sb[:],
                                     rhs=wt[:, k * D:(k + 1) * D],
                                     start=(k == 0), stop=(k == kc - 1))
                    nc.tensor.matmul(out=pl[:, :2 * h], lhsT=zt_sb[:],
                                     rhs=wat[:, k * 2 * h:(k + 1) * 2 * h],
                                     start=(k == 0), stop=(k == kc - 1))

                ptile = npool.tile([P, tcl], BF16, name="ptile", tag="ptb")
                nc.vector.tensor_tensor(
                    out=ptile[:, 0:D], in0=pp[:, :D],
                    in1=wsb[f'cv{li}'][:], op=OP.add)
                lsum = npool.tile([P, 2 * h], F32, name="lsum", tag="lsum")
                nc.vector.tensor_tensor(
                    out=lsum[:], in0=pl[:, :2 * h],
                    in1=wsb[f'c0{li}'][:], op=OP.add)
                nc.scalar.copy(out=ptile[:, D:D + h], in_=lsum[:, 0:h])
                nc.scalar.copy(out=adst_sb[:, t * 8:t * 8 + h],
                               in_=lsum[:, h:2 * h])
                nc.sync.dma_start(out=shard[li][rows, :], in_=ptile[:])

            def ag_group(li, j):
                if j == 0:
                    rows = slice(0, SPLIT * P)
                    ftab = fullA[li]
                else:
                    rows = slice(SPLIT * P, NB * P)
                    ftab = fullB[li]
                nc.gpsimd.collective_compute(
                    "AllGather", OP.bypass, replica_groups=rg,
                    ins=[shard[li][rows, :].opt()], outs=[ftab[:].opt()])

            # ---------------- per-block edge compute ----------------
            def edge_block(li, bl):
                ci, D, h, co, tcl = LAYERS[li]
                rows = slice(bl * P, (bl + 1) * P)
                g = bpool.tile([P, NCHB * tcl], BF16, name="g", tag="g")
                col = bl * NCHB * 8
                nc.gpsimd.dma_gather(
                    out_ap=g[:, 0:CA * tcl].rearrange("p (c t) -> p c t", c=CA),
                    in_ap=fullA[li][:],
                    idxs_ap=gidx[:, col:col + CA * 8],
                    num_idxs=CA * P, num_idxs_reg=CA * P, elem_size=tcl)
                nc.gpsimd.dma_gather(
                    out_ap=g[:, CA * tcl:].rearrange("p (c t) -> p c t", c=CB),
                    in_ap=fullB[li][:],
                    idxs_ap=gidx[:, col + CA * 8:col + NCHB * 8],
                    num_idxs=CB * P, num_idxs_reg=CB * P, elem_size=tcl)
                gv = g[:].rearrange("p (c t) -> p c t", c=NCHB)

                # one-hot S for all chunks (on-device, from didx)
                dcol = bl * NCHB
                s_all = epool.tile([P, NCHB * P], BF16, name="s_all", tag="S")
                sv = s_all[:].rearrange("p (c d) -> p c d", c=NCHB)
                nc.vector.tensor_tensor(
                    out=sv,
                    in0=didx[:, dcol:dcol + NCHB].unsqueeze(2)
                        .broadcast_to([P, NCHB, P]),
                    in1=iota_f[:].unsqueeze(1).broadcast_to([P, NCHB, P]),
                    op=OP.is_equal)

                # dst logits broadcast to edges: ade = S^T-matmul
                ade = psB.tile([P, NCHB * 8], F32, name="ade", tag="adps")
                for ch in range(NCHB):
                    t_ps = psT.tile([P, P], BF16, name="t_ps", tag="tp")
                    nc.tensor.transpose(out=t_ps[:],
                                        in_=s_all[:, ch * P:(ch + 1) * P],
                                        identity=ident_bf[:])
                    t_sb = epool.tile([P, P], BF16, name="t_sb", tag="T")
                    if ch % 2 == 0:
                        nc.scalar.copy(out=t_sb[:], in_=t_ps[:])
                    else:
                        nc.vector.tensor_copy(out=t_sb[:], in_=t_ps[:])
                    nc.tensor.matmul(
                        out=ade[:, ch * 8:ch * 8 + h], lhsT=t_sb[:],
                        rhs=adst_sb[:, bl * 8:bl * 8 + h],
                        start=True, stop=True)

                if _DBG and li == 0 and bl == 0:
                    nc.sync.dma_start(out=dbg['g0'][:], in_=g[:])
                    nc.sync.dma_start(out=dbg['s0'][:], in_=s_all[:])
                    _dump(nc, npool, dbg['ade0'], ade[:], NCHB * 8)

                # self-edge (local, not gathered): p_blk from own shard
                pself = epool.tile([P, tcl], BF16, name="pself", tag="pself")
                nc.sync.dma_start(out=pself[:], in_=shard[li][rows, :])
                es_raw = epool.tile([P, h], F32, name="es_raw", tag="esr")
                nc.vector.tensor_tensor(
                    out=es_raw[:], in0=pself[:, D:D + h],
                    in1=adst_sb[:, bl * 8:bl * 8 + h], op=OP.add)
                es_l = epool.tile([P, h], F32, name="es_l", tag="esl")
                nc.scalar.activation(out=es_l[:], in_=es_raw[:], func=AF.Prelu,
                                     alpha=NEG_SLOPE)
                exs = epool.tile([P, h], F32, name="exs", tag="exs")
                nc.scalar.activation(out=exs[:], in_=es_l[:], func=AF.Exp)

                # e = prelu(al_src + al_dst); exp(e) -> g logit slot
                eraw = epool.tile([P, NCHB * h], F32, name="eraw", tag="eraw")
                nc.vector.tensor_tensor(
                    out=eraw[:].rearrange("p (c h) -> p c h", c=NCHB),
                    in0=gv[:, :, D:D + h],
                    in1=ade[:].rearrange("p (c h) -> p c h", c=NCHB)[:, :, 0:h],
                    op=OP.add)
                el = epool.tile([P, NCHB * h], F32, name="el", tag="el")
                nc.scalar.activation(out=el[:], in_=eraw[:], func=AF.Prelu,
                                     alpha=NEG_SLOPE)
                nc.scalar.activation(
                    out=gv[:, :, D:D + h],
                    in_=el[:].rearrange("p (c h) -> p c h", c=NCHB),
                    func=AF.Exp)

                # weight messages in place per chunk
                for ch in range(NCHB):
                    ms = g[:, ch * tcl:ch * tcl + D]
                    nc.vector.tensor_tensor(
                        out=ms.rearrange("p (h c) -> p h c", h=h),
                        in0=ms.rearrange("p (h c) -> p h c", h=h),
                        in1=g[:, ch * tcl + D:ch * tcl + D + h]
                            .unsqueeze(2).broadcast_to([P, h, co]),
                        op=OP.mult)

                # scatter-add via one-hot matmuls
                pa = psA.tile([P, 512], F32, name="pa", tag="mmA")
                pb = psB.tile([P, 16], F32, name="pb", tag="mmB")
                for ch in range(NCHB):
                    nc.tensor.matmul(
                        out=pa[:, :D], lhsT=s_all[:, ch * P:(ch + 1) * P],
                        rhs=g[:, ch * tcl:ch * tcl + D],
                        start=(ch == 0), stop=(ch == NCHB - 1))
                    nc.tensor.matmul(
                        out=pb[:, :h], lhsT=s_all[:, ch * P:(ch + 1) * P],
                        rhs=g[:, ch * tcl + D:ch * tcl + D + h],
                        start=(ch == 0), stop=(ch == NCHB - 1))
                if _DBG and li == 0 and bl == 0:
                    _dump(nc, npool, dbg['pa0'], pa[:, :512], 512)
                    _dump(nc, npool, dbg['pb0'], pb[:, :16], 16)
                dsum = spool.tile([P, h], F32, name="dsum", tag="ds")
                nc.vector.tensor_tensor(out=dsum[:], in0=pb[:, :h], in1=exs[:],
                                        op=OP.add)
                rs = spool.tile([P, h], F32, name="rs", tag="rs")
                nc.vector.reciprocal(out=rs[:], in_=dsum[:])
                nsum = npool.tile([P, D], F32, name="nsum", tag="nsum")
                if h > 1:
                    nc.vector.tensor_tensor(
                        out=nsum[:].rearrange("p (h c) -> p h c", h=h),
                        in0=pself[:, 0:D].rearrange("p (h c) -> p h c", h=h),
                        in1=exs[:].unsqueeze(2).broadcast_to([P, h, co]),
                        op=OP.mult)
                else:
                    nc.vector.tensor_scalar_mul(nsum[:], pself[:, 0:D],
                                                exs[:, 0:1])
                nc.vector.tensor_tensor(out=nsum[:], in0=nsum[:],
                                        in1=pa[:, :D], op=OP.add)
                gat = npool.tile([P, D], F32, name="gat", tag="gat")
                if h > 1:
                    nc.vector.tensor_tensor(
                        out=gat[:].rearrange("p (h c) -> p h c", h=h),
                        in0=nsum[:].rearrange("p (h c) -> p h c", h=h),
                        in1=rs[:].unsqueeze(2).broadcast_to([P, h, co]),
                        op=OP.mult)
                else:
                    nc.vector.tensor_scalar_mul(gat[:], nsum[:], rs[:, 0:1])

                bias_b = wsb[f'bias{li}'][:]
                hrow = h_sb[:, bl * 512:(bl + 1) * 512]
                if li < 3:
                    t0 = npool.tile([P, D], F32, name="t0", tag="t0")
                    nc.vector.tensor_tensor(out=t0[:], in0=gat[:], in1=bias_b,
                                            op=OP.add)
                    if li == 0:
                        nc.scalar.activation(out=hrow, in_=t0[:], func=AF.Gelu)
                    else:
                        g1 = npool.tile([P, D], F32, name="g1", tag="g1")
                        nc.scalar.activation(out=g1[:], in_=t0[:], func=AF.Gelu)
                        nc.vector.tensor_tensor(out=hrow, in0=g1[:], in1=hrow,
                                                op=OP.add)
                else:
                    of = npool.tile([P, OUT], F32, name="of", tag="of")
                    nc.vector.tensor_tensor(out=of[:], in0=gat[:], in1=bias_b,
                                            op=OP.add)
                    nc.vector.tensor_tensor(
                        out=of[:], in0=of[:],
                        in1=res_sb[:, bl * OUT:(bl + 1) * OUT], op=OP.add)
                    nc.sync.dma_start(out=out_ext[rows, :], in_=of[:])

            # ---------------- schedule ----------------
            def run_groups(fn):
                for t in range(NB):
                    fn(t)
                    if t == SPLIT - 1:
                        yield 0
                    elif t == NB - 1:
                        yield 1

            for li in range(4):
                for j in run_groups(lambda t, _li=li: node_block(_li, t)):
                    ag_group(li, j)
                for bl in range(NB):
                    edge_block(li, bl)

    nc.compile()
    return nc


# ------------------------------------------------------------------- kernel

_CACHE = {}


def kernel(**inputs):
    inputs = {k: np.asarray(v) for k, v in inputs.items()}
    new_id, gidx, didx, CA, CB = _build_partition(inputs['edge_index'])
    ws = _fold_weights(inputs)

    x = np.asarray(inputs['x'], np.float32)
    xp = np.zeros((NTOT, IN), np.float32)
    xp[new_id] = x

    key = (CA, CB)
    if key not in _CACHE:
        _CACHE[key] = _build_program(CA, CB)
    nc = _CACHE[key]

    in_maps = []
    for c in range(NCORES):
        m = dict(
            x_s=np.ascontiguousarray(xp[c * NPC:(c + 1) * NPC]),
            gidx=np.ascontiguousarray(gidx[c]),
            didx=np.ascontiguousarray(didx[c]),
        )
        m.update(ws)
        in_maps.append(m)

    res = run_bass_kernel_spmd(nc, in_maps, core_ids=list(range(NCORES)))
    global _LAST_RES
    _LAST_RES = res
    outs = np.concatenate([r["out"] for r in res.results], axis=0)  # [NTOT, 3]
    return np.ascontiguousarray(outs[new_id])


_LAST_RES = None
